# revision 28
# baseline (speedup 1.0000x reference)
"""GCN layer (CustomGraphConv) on 8 trn2 NeuronCores via Bass/Tile.

Math: out = D^{-1/2} (A + I) D^{-1/2} @ X @ W + bias
  - A: [N, N] 0/1 symmetric adjacency (f32 input), N = 8192
  - X: [N, 256] f32, W: [256, 256] f32, bias: [256] f32

Sharding: 1D node partition. Core c owns nodes R_c = [c*C, (c+1)*C), C = N/8.
Each core receives the column strip A_aug[:, R_c] (A with self-loops added on
the host, cast to fp8e4 — exact for 0/1 values, pre-tiled partition-major so
pack loads are single-descriptor-class DMAs), the full X^T and W in fp16
(replicated), and the bias broadcast to a [128, 256] f32 tile.

Device program (identical SPMD program on all 8 cores):
  1. Load A strip as 8 packed [128, 8C] fp8 tiles (A read from HBM once;
     big contiguous packs keep the load bytes-bound). X^T chunks are
     issued on the same queue AFTER the A packs so the A load (which
     gates the degree collective -> critical path) is never delayed.
  2. deg = colsum(strip) on PE in fp8 DoubleRow perf mode: ones[128,2,128]
     stationary x A k-tile pairs moving, 2 contraction subtiles per pass.
     Row 0 of each psum chunk = degrees of own nodes (complete, by
     symmetry of A). One accumulation group per PSUM bank.
  3. AllGather degree shards -> deg_full [N]; shards are stored p-major so
     the gathered tensor DMAs back into [128, KT] with 32B-contiguous
     descriptors. ds = S/sqrt(deg) (reciprocal + scaled sqrt), folding the
     fp8 block scale S; d_own' = (1/S)/sqrt(deg_own) for the epilogue.
  4. Z = X @ W via PE in fp16 (X^T chunks stationary, W moving). This has
     no dependency on the degrees, so it executes entirely inside the
     collective's latency window.
  5. Zd = ds * Z per-partition on ACT, then split to fp8 hi/lo pairs:
     hi = fp8(Zd) (DVE cast), lo = fp8(Zd - hi) (DVE/Pool subtract).
     hi + lo carries ~2^-8 relative error; S=128 keeps values in e4m3's
     normal range.
  6. out_psum[jt] += A_pair[k]^T @ Zhi_pair[k] + A_pair[k]^T @ Zlo_pair[k]
     via fp8 DoubleRow matmuls, k-outer / jt-inner: 8 simultaneous
     accumulation groups, one per PSUM bank, so pairs are consumed as the
     hi/lo conversion pipeline produces them.
  7. out = d_own' * out_psum + bias; per-jt stores.

Toolchain constraints discovered on this stack:
  - walrus rejects >1 semaphore wait per instruction -> _split_dma_waits
    hoists extras onto standalone EventSemaphore instructions.
  - SBUF access patterns must keep the partition dim explicit: t[0, :]
    (rank-dropped) misdrives the DMA; use t[0:1, :].
  - A matmul accumulation group must own its PSUM bank exclusively until
    `stop` (start=True clears the whole bank) -> PSUM pools are scoped per
    phase (4-bank colsum pool closes before the 8-bank output pool opens).
  - fp8 is exact for 0/1 adjacency values; fp8 DoubleRow matmuls contract
    two 128-row subtiles per pass (both operands must be fp8e4/e5).
"""

import numpy as np
import ml_dtypes

import concourse.bass as bass
import concourse.mybir as mybir
import concourse.tile as tile
from concourse.bass_utils import run_bass_kernel_spmd

NCORES = 8
F = 256
S = 128.0  # fp8 block scale for the Zd hi/lo split

f32 = mybir.dt.float32
fp16 = mybir.dt.float16
fp8 = mybir.dt.float8e4
DR = mybir.MatmulPerfMode.DoubleRow


def _split_dma_waits(nc):
    """Hoist semaphore waits onto standalone EventSemaphore instructions on
    the issuing engine's queue, for any instruction carrying more than one.

    This toolchain's walrus caps sync waits at 1 per instruction (2 for
    EventSemaphore). A sequencer executes an attached wait and a preceding
    standalone wait identically, so hoisting preserves semantics (raw-bass
    wait_ge emits exactly this instruction).
    """
    ctr = 0
    for fn in nc.m.functions:
        for bb in fn.blocks:
            new_insts = []
            for inst in bb.instructions:
                si = inst.sync_info
                if (
                    not isinstance(inst, mybir.InstEventSemaphore)
                    and si is not None
                    and len(si.on_wait) > 1
                ):
                    for w in si.on_wait[:-1]:
                        ev = mybir.InstEventSemaphore(
                            name=f"hoistw-{ctr}",
                            engine=inst.engine,
                            ins=[],
                            outs=[],
                            sync_info=mybir.SyncInfo(on_wait=[w], on_update=[]),
                        )
                        ctr += 1
                        new_insts.append(ev)
                    inst.sync_info = mybir.SyncInfo(
                        on_wait=[si.on_wait[-1]], on_update=si.on_update
                    )
                new_insts.append(inst)
            bb.instructions = new_insts


def build(n_nodes: int, debug: bool = False, split_waits: bool = True):
    """Build the SPMD Bass program for one core (all cores identical)."""
    N = n_nodes
    C = N // NCORES  # own nodes per core
    KT = N // 128  # 128-row k tiles of the strip
    NP = KT // 2  # k-tile pairs (DoubleRow contracts 2 per matmul)
    JT = C // 128  # 128-col j tiles (own-node blocks)
    J = KT // NCORES  # p-major degree columns per core (== JT)
    # k-tiles per A-load DMA (even). Few big packs amortize per-DMA gaps on
    # the exclusive DMA device; the tail is split fine so the last colsum
    # burst (on the critical path into the collective) is short.
    if KT == 64:
        packs = [16, 16, 16, 8, 4, 2, 2]
    else:
        packs = [KT]
    assert sum(packs) == KT and all(p % 2 == 0 for p in packs)
    DEG_W = min(C, 256)  # colsum chunk width (DoubleRow moving free <= 512)
    DEG_CH = C // DEG_W
    XCOLS = min(2048, N // 2)  # X^T columns per load chunk
    XCH = N // XCOLS
    MPC = XCOLS // 128  # m tiles per X^T chunk

    nc = bass.Bass()
    # partition-major pre-tiled strip: a_strip[p, k*C + c] = A_aug[k*128+p, own c]
    a_strip = nc.dram_tensor("a_strip", [128, KT * C], fp8, kind="ExternalInput")
    xt = nc.dram_tensor("xt", [F, N], fp16, kind="ExternalInput")
    w = nc.dram_tensor("w", [F, F], fp16, kind="ExternalInput")
    bias_bc = nc.dram_tensor("bias_bc", [128, F], f32, kind="ExternalInput")
    out = nc.dram_tensor("out", [C, F], f32, kind="ExternalOutput")

    with tile.TileContext(nc) as tc:
        with (
            tc.tile_pool(name="persist", bufs=1) as persist,
            tc.tile_pool(name="work", bufs=2) as work,
            tc.tile_pool(name="dram", bufs=1, space="DRAM") as dram,
        ):
            # ---- A strip loads: big contiguous packs, all on the SP queue
            # first (nothing else may delay A — it gates the degree
            # collective, which gates everything after it).
            a_pk = []
            k2pack = []  # k-tile -> (pack idx, offset within pack)
            k0 = 0
            for g, pk in enumerate(packs):
                t = persist.tile([128, pk * C], fp8, name=f"a{g}")
                nc.sync.dma_start(
                    out=t[:], in_=a_strip[:, k0 * C : (k0 + pk) * C]
                )
                a_pk.append(t)
                for i in range(pk):
                    k2pack.append((g, i))
                k0 += pk

            def a_pair(kp, c0, c1):
                """[128, 2, c1-c0] fp8 AP of k-tile pair kp, cols [c0, c1)."""
                g, i = k2pack[2 * kp]
                v = a_pk[g][:, i * C : (i + 2) * C].rearrange(
                    "p (t c) -> p t c", t=2
                )
                return v[:, :, c0:c1]

            # small replicated params: same SP queue, directly after A
            w_sb = [persist.tile([128, F], fp16, name=f"w{i}") for i in range(2)]
            for i in range(2):
                nc.sync.dma_start(out=w_sb[i][:], in_=w[i * 128 : (i + 1) * 128, :])
            bias_sb = persist.tile([128, F], f32, name="bias")
            nc.sync.dma_start(out=bias_sb[:], in_=bias_bc[:])

            # X^T chunk loads: same SP queue, AFTER the A packs.
            xt_tiles = {}
            for ch in range(XCH):
                for half in range(2):
                    t = work.tile(
                        [128, XCOLS],
                        fp16,
                        name=f"xt_{ch}_{half}",
                        tag=f"xt{half}",
                        bufs=2,
                    )
                    nc.sync.dma_start(
                        out=t[:],
                        in_=xt[
                            half * 128 : (half + 1) * 128,
                            ch * XCOLS : (ch + 1) * XCOLS,
                        ],
                    )
                    xt_tiles[(ch, half)] = t

            ones = persist.tile([128, 256], fp8, name="ones")
            nc.vector.memset(ones[:], 1.0)
            ones_v = ones.rearrange("p (t m) -> p t m", t=2)

            # ---- degrees of own nodes: colsum of the strip via fp8
            # DoubleRow. ones (stationary) x A pairs (moving); any psum row
            # = colsums. One accumulation group per bank; the pool closes
            # (banks freed) before the later psum pools open. The colsum
            # chunks drain straight to DRAM (cc_in, p-major) for the
            # collective — no SBUF bounce.
            cc_in = dram.tile([C], f32, name="cc_in")
            cc_out = dram.tile([N], f32, name="cc_out")
            last_cs_mm = None
            with tc.tile_pool(name="degpsum", bufs=1, space="PSUM") as degpsum:
                deg_ps = [
                    degpsum.tile([128, DEG_W], f32, name=f"deg_ps{h}")
                    for h in range(DEG_CH)
                ]
                for kp in range(NP):
                    for h in range(DEG_CH):
                        last_cs_mm = nc.tensor.matmul(
                            deg_ps[h][:],
                            ones_v[:, :, 0:128],
                            a_pair(kp, h * DEG_W, (h + 1) * DEG_W),
                            start=(kp == 0),
                            stop=(kp == NP - 1),
                            perf_mode=DR,
                        )
                # drain chunk row 0s to SBUF (alternating engines), then one
                # DMA ships all degrees to the DRAM collective input,
                # scattered p-major: cc_in[p*J + j] = deg(j*128 + p).
                deg_sb = persist.tile([1, C], f32, name="deg_sb")
                for h in range(DEG_CH):
                    seg = deg_sb[:, h * DEG_W : (h + 1) * DEG_W]
                    if h % 2 == 0:
                        nc.vector.tensor_copy(seg, deg_ps[h][0:1, :])
                    else:
                        nc.scalar.copy(seg, deg_ps[h][0:1, :])
                nc.scalar.dma_start(
                    out=cc_in.rearrange("(p j) -> j p", p=128),
                    in_=deg_sb[0:1, :],
                )

            # ---- gather degrees (p-major shards so both readbacks are
            # contiguous-ish), d = scaled rsqrt ----
            coll = nc.gpsimd.collective_compute(
                "AllGather",
                mybir.AluOpType.bypass,
                replica_groups=[list(range(NCORES))],
                ins=[cc_in[:]],
                outs=[cc_out[:]],
            )
            deg_full = work.tile([128, KT], f32, tag="deg_full")
            nc.sync.dma_start(
                out=deg_full.rearrange("p (c j) -> p c j", c=NCORES),
                in_=cc_out.rearrange("(c p j) -> p c j", p=128, c=NCORES),
            )
            # ds = S / sqrt(deg): reciprocal then sqrt(S^2 * x)
            ds = persist.tile([128, KT], f32, name="ds")
            ds_recip = nc.vector.reciprocal(ds[:], deg_full[:])
            nc.scalar.activation(
                ds[:], ds[:], mybir.ActivationFunctionType.Sqrt, scale=S * S
            )

            # d_own' = (1/S) / sqrt(deg_own), from the local (pre-gather) degs
            deg_own = work.tile([128, J], f32, tag="deg_own")
            nc.sync.dma_start(
                out=deg_own[:], in_=cc_in.rearrange("(p j) -> p j", p=128)
            )
            d_own = persist.tile([128, J], f32, name="d_own")
            nc.vector.reciprocal(d_own[:], deg_own[:])
            nc.scalar.activation(
                d_own[:],
                d_own[:],
                mybir.ActivationFunctionType.Sqrt,
                scale=1.0 / (S * S),
            )

            # ---- Z = X @ W (fp16 in, f32 accum, fp16 out), unscaled.
            # No degree dependency: fills the collective's latency window.
            # 4 psum bufs so the PE never stalls on the psum->SBUF drains.
            z_sb = [persist.tile([128, F], fp16, name=f"z{m}") for m in range(KT)]
            first_z_mm = None
            last_xw_mm = None
            with tc.tile_pool(name="zpsum", bufs=6, space="PSUM") as zpsum:
                for ch in range(XCH):
                    for mi in range(MPC):
                        m = ch * MPC + mi
                        z_ps = zpsum.tile([128, F], f32, tag="z_ps")
                        for i in range(2):
                            mm = nc.tensor.matmul(
                                z_ps[:],
                                xt_tiles[(ch, i)][:, mi * 128 : (mi + 1) * 128],
                                w_sb[i][:],
                                start=(i == 0),
                                stop=(i == 1),
                            )
                            last_xw_mm = mm
                            if first_z_mm is None:
                                first_z_mm = mm
                                # keep the Z matmuls behind the colsum chain
                                # on the PE queue (colsum gates the
                                # collective -> critical path)
                                bass._add_dep_helper(
                                    mm.ins, last_cs_mm.ins, reason="z after colsum"
                                )
                        if m % 2 == 0:
                            nc.vector.tensor_copy(z_sb[m][:], z_ps[:])
                        else:
                            nc.scalar.copy(z_sb[m][:], z_ps[:])

            # ---- fp8 hi/lo split of Zd = ds * Z, two fused ops per tile:
            #   hi = fp8(z * ds)        (ACT activation-with-scale / DVE
            #                            tensor_scalar, fp8 out)
            #   lo = fp8(z * ds - hi)   (DVE/Pool scalar_tensor_tensor)
            # rotated so no single engine becomes the pipeline bottleneck.
            zhi = [persist.tile([128, 2 * F], fp8, name=f"zh{kp}") for kp in range(NP)]
            zlo = [persist.tile([128, 2 * F], fp8, name=f"zl{kp}") for kp in range(NP)]
            for m in range(KT):
                kp, half = divmod(m, 2)
                hi = zhi[kp][:, half * F : (half + 1) * F]
                lo = zlo[kp][:, half * F : (half + 1) * F]
                dsm = ds[:, m : m + 1]
                nc.scalar.activation(
                    hi,
                    z_sb[m][:],
                    mybir.ActivationFunctionType.Copy,
                    scale=dsm,
                )
                # walrus: TensorScalarPtr is DVE-only (Pool rejected at
                # codegen) -> all lo ops ride DVE
                nc.vector.scalar_tensor_tensor(
                    lo,
                    z_sb[m][:],
                    dsm,
                    hi,
                    mybir.AluOpType.mult,
                    mybir.AluOpType.subtract,
                )

            # ---- big matmul: out[j, f] += A[i, j] * Zd[i, f] in fp8
            # DoubleRow, k-outer / jt-inner: 8 simultaneous accumulation
            # groups, one per PSUM bank, consuming hi/lo pairs as the
            # conversion pipeline produces them.
            with tc.tile_pool(name="outpsum", bufs=1, space="PSUM") as outpsum:
                out_ps = [
                    outpsum.tile([128, F], f32, name=f"ops{jt}") for jt in range(JT)
                ]
                # PE p-state warm-up across the PE-idle gap between the
                # collective's end and the first (degree-gated) fp8 pair:
                # one dummy chain fires at the collective's completion, a
                # second at the ds reciprocal, so the PE is at full clock
                # when the real pairs arrive (they queue behind on the PE
                # FIFO, so overshoot would delay the MM — keep them short).
                for i in range(48):
                    dmm = nc.tensor.matmul(
                        out_ps[0][:, 0:128],
                        ones_v[:, :, 0:128],
                        ones_v[:, :, 0:128],
                        start=True,
                        stop=True,
                        perf_mode=DR,
                    )
                    dep = coll if i < 24 else ds_recip
                    bass._add_dep_helper(dmm.ins, dep.ins, reason="pe warmup")
                for kp in range(NP):
                    zh = zhi[kp].rearrange("p (t f) -> p t f", t=2)
                    zl = zlo[kp].rearrange("p (t f) -> p t f", t=2)
                    for jt in range(JT):
                        ap = a_pair(kp, jt * 128, (jt + 1) * 128)
                        nc.tensor.matmul(
                            out_ps[jt][:],
                            ap,
                            zh,
                            start=(kp == 0),
                            stop=False,
                            perf_mode=DR,
                        )
                        nc.tensor.matmul(
                            out_ps[jt][:],
                            ap,
                            zl,
                            start=False,
                            stop=(kp == NP - 1),
                            perf_mode=DR,
                        )
                        if kp == NP - 1:
                            # epilogue pipelined into the final pair:
                            # out = d_own' * psum + bias, spread across
                            # DVE (fused) and ACT+Pool (scale, then add)
                            # so the tail drains in parallel.
                            # later jts get the short DVE->SP chain so the
                            # final store (the kernel's tail) drains fastest
                            ot = work.tile([128, F], f32, tag="ot", bufs=8)
                            if jt % 2 == 1:
                                nc.vector.scalar_tensor_tensor(
                                    ot[:],
                                    out_ps[jt][:],
                                    d_own[:, jt : jt + 1],
                                    bias_sb[:],
                                    mybir.AluOpType.mult,
                                    mybir.AluOpType.add,
                                )
                            else:
                                sc = work.tile([128, F], f32, tag="sc", bufs=4)
                                nc.scalar.activation(
                                    sc[:],
                                    out_ps[jt][:],
                                    mybir.ActivationFunctionType.Copy,
                                    scale=d_own[:, jt : jt + 1],
                                )
                                nc.gpsimd.tensor_tensor(
                                    ot[:], sc[:], bias_sb[:], mybir.AluOpType.add
                                )
                            eng = nc.sync if jt % 2 == 1 else nc.scalar
                            eng.dma_start(
                                out=out[jt * 128 : (jt + 1) * 128, :], in_=ot[:]
                            )
    if split_waits:
        _split_dma_waits(nc)
    return nc


_CACHE = {}


def _get_program(n_nodes: int, debug: bool = False):
    key = (n_nodes, debug)
    if key not in _CACHE:
        _CACHE[key] = build(n_nodes, debug=debug)
    return _CACHE[key]


def _prep_inputs(A, inputs, weight, bias):
    """Host-side marshaling: shard + layout + dtype casts."""
    N = A.shape[0]
    C = N // NCORES
    KT = N // 128
    A_aug = np.asarray(A, dtype=np.float32)
    idx = np.arange(N)
    A_aug = A_aug.astype(ml_dtypes.float8_e4m3)
    A_aug[idx, idx] = np.float32(1.0)  # reference adds I; A diag is 0
    xt = np.ascontiguousarray(np.asarray(inputs, dtype=np.float32).T).astype(
        np.float16
    )
    w16 = np.asarray(weight, dtype=np.float32).astype(np.float16)
    bias_bc = np.ascontiguousarray(
        np.broadcast_to(np.asarray(bias, dtype=np.float32), (128, F))
    )
    in_maps = []
    for c in range(NCORES):
        strip = A_aug[:, c * C : (c + 1) * C]  # [N, C]
        # partition-major pre-tiling: [128, KT*C]
        tiled = np.ascontiguousarray(
            strip.reshape(KT, 128, C).transpose(1, 0, 2).reshape(128, KT * C)
        )
        in_maps.append(
            {"a_strip": tiled, "xt": xt, "w": w16, "bias_bc": bias_bc}
        )
    return in_maps


def kernel(A, inputs, weight, bias):
    N = A.shape[0]
    nc = _get_program(N)
    in_maps = _prep_inputs(A, inputs, weight, bias)
    res = run_bass_kernel_spmd(nc, in_maps, list(range(NCORES)))
    return np.concatenate([r["out"] for r in res.results], axis=0)


if __name__ == "__main__":
    # mini self-check with a host reference
    N = 1024
    rng = np.random.default_rng(0)
    A = (rng.random((N, N)) < 0.01).astype(np.float32)
    A = np.maximum(A, A.T)
    np.fill_diagonal(A, 0.0)
    X = rng.standard_normal((N, F)).astype(np.float32)
    W = (rng.random((F, F)).astype(np.float32) / 100.0) - 0.005
    b = (rng.random(F).astype(np.float32) / 100.0) - 0.005

    A_ = A + np.eye(N, dtype=np.float32)
    deg = A_.sum(axis=1)
    d = deg**-0.5
    expected = (d[:, None] * A_ * d[None, :]) @ X @ W + b

    nc = _get_program(N)
    in_maps = _prep_inputs(A, X, W, b)
    res = run_bass_kernel_spmd(nc, in_maps, list(range(NCORES)))
    got = np.concatenate([r["out"] for r in res.results], axis=0)
    err = np.abs(got - expected)
    scale = np.abs(expected).max()
    print("rel err:", err.max() / scale, "nan:", np.isnan(got).sum(), "/", got.size)


# revision 37
# speedup vs baseline: 1.0366x; 1.0366x over previous
"""GCN layer (CustomGraphConv) on 8 trn2 NeuronCores via Bass/Tile.

Math: out = D^{-1/2} (A + I) D^{-1/2} @ X @ W + bias
  - A: [N, N] 0/1 symmetric adjacency (f32 input), N = 8192
  - X: [N, 256] f32, W: [256, 256] f32, bias: [256] f32

Sharding: 1D node partition. Core c owns nodes R_c = [c*C, (c+1)*C), C = N/8.
Each core receives the column strip A_aug[:, R_c] (A with self-loops added on
the host, cast to fp8e4 — exact for 0/1 values, pre-tiled partition-major so
pack loads are single-descriptor-class DMAs), the full X^T and W in fp16
(replicated), and the bias broadcast to a [128, 256] f32 tile.

Device program (identical SPMD program on all 8 cores):
  1. Load A strip as packed fp8 tiles, sizes [32,16,8,4,2,2] k-tiles: big
     leading packs amortize per-DMA overhead on the (exclusive) DMA path,
     the fine tail keeps the last colsum burst short — it gates the
     collective, which gates everything after it. w/bias then X^T chunks
     are issued on the same queue AFTER the A packs so A is never delayed.
  2. deg = colsum(strip) on PE in fp8 DoubleRow perf mode: ones[128,2,128]
     stationary x A k-tile pairs moving, 2 contraction subtiles per pass
     at 0.5 cycles/row. Row 0 of each psum chunk = degrees of own nodes
     (complete, by symmetry of A). One accumulation group per PSUM bank;
     the colsum pool closes before the 8-bank output pool opens.
  3. AllGather degree shards -> deg_full [N]; shards are stored p-major so
     the gathered tensor DMAs back into [128, KT] with 32B-contiguous
     descriptors. ds = S/sqrt(deg) (reciprocal + scaled sqrt), folding the
     fp8 block scale S; d_own' = (1/S)/sqrt(deg_own) for the epilogue.
     An early 1-element Sqrt activation preloads the ACT function table
     (Copy+Sqrt set) so no table load lands on the critical path.
  4. Z = X @ W via PE in fp16 (X^T chunks stationary, W moving). This has
     no dependency on the degrees, so it executes entirely inside the
     collective's ~16us latency window (6 psum bufs so the PE never
     stalls on the alternating DVE/ACT drains).
  5. Zd split to fp8 hi/lo pairs, two fused ops per tile:
     hi = fp8(ds*Z) (ACT activation-with-scale), lo = fp8(ds*Z - hi)
     (DVE scalar_tensor_tensor; walrus rejects TensorScalarPtr on Pool).
     hi + lo carries ~2^-8 relative error; S=128 keeps values in e4m3's
     normal range.
  6. out_psum[jt] += A_pair[k]^T @ Zhi_pair[k] + A_pair[k]^T @ Zlo_pair[k]
     via fp8 DoubleRow matmuls, k-outer / jt-inner: 8 simultaneous
     accumulation groups, one per PSUM bank, consuming pairs as the
     conversion pipeline produces them. Dummy ones x ones matmuls chained
     on the collective / reciprocal keep the PE p-state ramped across the
     degree-gated idle gap.
  7. out = d_own' * out_psum + bias, pipelined into the final pair:
     alternating DVE-fused / ACT+Pool epilogues, stores split SP/ACT.

Toolchain constraints discovered on this stack:
  - walrus rejects >1 semaphore wait per instruction -> _split_dma_waits
    hoists extras onto standalone EventSemaphore instructions.
  - SBUF access patterns must keep the partition dim explicit: t[0, :]
    (rank-dropped) misdrives the DMA; use t[0:1, :].
  - A matmul accumulation group must own its PSUM bank exclusively until
    `stop` (start=True clears the whole bank) -> PSUM pools are scoped per
    phase (4-bank colsum pool closes before the 8-bank output pool opens).
  - fp8 is exact for 0/1 adjacency values; fp8 DoubleRow matmuls contract
    two 128-row subtiles per pass (both operands must be fp8e4/e5).
"""

import numpy as np
import ml_dtypes

import concourse.bass as bass
import concourse.mybir as mybir
import concourse.tile as tile
from concourse.bass_utils import run_bass_kernel_spmd

NCORES = 8
F = 256
S = 128.0  # fp8 block scale for the Zd hi/lo split

f32 = mybir.dt.float32
fp16 = mybir.dt.float16
fp8 = mybir.dt.float8e4
DR = mybir.MatmulPerfMode.DoubleRow


def _split_dma_waits(nc):
    """Hoist semaphore waits onto standalone EventSemaphore instructions on
    the issuing engine's queue, for any instruction carrying more than one.

    This toolchain's walrus caps sync waits at 1 per instruction (2 for
    EventSemaphore). A sequencer executes an attached wait and a preceding
    standalone wait identically, so hoisting preserves semantics (raw-bass
    wait_ge emits exactly this instruction).
    """
    ctr = 0
    for fn in nc.m.functions:
        for bb in fn.blocks:
            new_insts = []
            for inst in bb.instructions:
                si = inst.sync_info
                if (
                    not isinstance(inst, mybir.InstEventSemaphore)
                    and si is not None
                    and len(si.on_wait) > 1
                ):
                    for w in si.on_wait[:-1]:
                        ev = mybir.InstEventSemaphore(
                            name=f"hoistw-{ctr}",
                            engine=inst.engine,
                            ins=[],
                            outs=[],
                            sync_info=mybir.SyncInfo(on_wait=[w], on_update=[]),
                        )
                        ctr += 1
                        new_insts.append(ev)
                    inst.sync_info = mybir.SyncInfo(
                        on_wait=[si.on_wait[-1]], on_update=si.on_update
                    )
                new_insts.append(inst)
            bb.instructions = new_insts


def build(n_nodes: int, debug: bool = False, split_waits: bool = True):
    """Build the SPMD Bass program for one core (all cores identical)."""
    N = n_nodes
    C = N // NCORES  # own nodes per core
    KT = N // 128  # 128-row k tiles of the strip
    NP = KT // 2  # k-tile pairs (DoubleRow contracts 2 per matmul)
    JT = C // 128  # 128-col j tiles (own-node blocks)
    J = KT // NCORES  # p-major degree columns per core (== JT)
    # k-tiles per A-load DMA (even). Few big packs amortize per-DMA gaps on
    # the exclusive DMA device; the tail is split fine so the last colsum
    # burst (on the critical path into the collective) is short.
    if KT == 64:
        packs = [32, 16, 8, 4, 2, 2]
    else:
        packs = [KT]
    assert sum(packs) == KT and all(p % 2 == 0 for p in packs)
    DEG_W = min(C, 256)  # colsum chunk width (DoubleRow moving free <= 512)
    DEG_CH = C // DEG_W
    XCOLS = min(2048, N // 2)  # X^T columns per load chunk
    XCH = N // XCOLS
    MPC = XCOLS // 128  # m tiles per X^T chunk

    nc = bass.Bass()
    # partition-major pre-tiled strip: a_strip[p, k*C + c] = A_aug[k*128+p, own c]
    a_strip = nc.dram_tensor("a_strip", [128, KT * C], fp8, kind="ExternalInput")
    xt = nc.dram_tensor("xt", [F, N], fp16, kind="ExternalInput")
    w = nc.dram_tensor("w", [F, F], fp16, kind="ExternalInput")
    bias_bc = nc.dram_tensor("bias_bc", [128, F], f32, kind="ExternalInput")
    out = nc.dram_tensor("out", [C, F], f32, kind="ExternalOutput")

    with tile.TileContext(nc) as tc:
        with (
            tc.tile_pool(name="persist", bufs=1) as persist,
            tc.tile_pool(name="work", bufs=2) as work,
            tc.tile_pool(name="dram", bufs=1, space="DRAM") as dram,
        ):
            # ---- A strip loads: big contiguous packs, all on the SP queue
            # first (nothing else may delay A — it gates the degree
            # collective, which gates everything after it).
            a_pk = []
            k2pack = []  # k-tile -> (pack idx, offset within pack)
            k0 = 0
            for g, pk in enumerate(packs):
                t = persist.tile([128, pk * C], fp8, name=f"a{g}")
                nc.sync.dma_start(
                    out=t[:], in_=a_strip[:, k0 * C : (k0 + pk) * C]
                )
                a_pk.append(t)
                for i in range(pk):
                    k2pack.append((g, i))
                k0 += pk

            def a_pair(kp, c0, c1):
                """[128, 2, c1-c0] fp8 AP of k-tile pair kp, cols [c0, c1)."""
                g, i = k2pack[2 * kp]
                v = a_pk[g][:, i * C : (i + 2) * C].rearrange(
                    "p (t c) -> p t c", t=2
                )
                return v[:, :, c0:c1]

            # small replicated params: same SP queue, directly after A
            w_sb = [persist.tile([128, F], fp16, name=f"w{i}") for i in range(2)]
            for i in range(2):
                nc.sync.dma_start(out=w_sb[i][:], in_=w[i * 128 : (i + 1) * 128, :])
            bias_sb = persist.tile([128, F], f32, name="bias")
            nc.sync.dma_start(out=bias_sb[:], in_=bias_bc[:])

            # X^T chunk loads: same SP queue, AFTER the A packs.
            xt_tiles = {}
            for ch in range(XCH):
                for half in range(2):
                    t = work.tile(
                        [128, XCOLS],
                        fp16,
                        name=f"xt_{ch}_{half}",
                        tag=f"xt{half}",
                        bufs=2,
                    )
                    nc.sync.dma_start(
                        out=t[:],
                        in_=xt[
                            half * 128 : (half + 1) * 128,
                            ch * XCOLS : (ch + 1) * XCOLS,
                        ],
                    )
                    xt_tiles[(ch, half)] = t

            ones = persist.tile([128, 256], fp8, name="ones")
            nc.vector.memset(ones[:], 1.0)
            ones_v = ones.rearrange("p (t m) -> p t m", t=2)

            # ACT activation-table warmers: the first Activation pays a
            # ~1.3us table load; fire tiny ones early (off the critical
            # path) so the degree drains / sqrt don't.
            # (the sqrt table set also contains Copy, so one warmer covers
            # every ACT op in this kernel)
            act_warm = persist.tile([1, 1], f32, name="act_warm")
            nc.scalar.activation(
                act_warm[:], ones[0:1, 0:1], mybir.ActivationFunctionType.Sqrt
            )

            # ---- degrees of own nodes: colsum of the strip via fp8
            # DoubleRow. ones (stationary) x A pairs (moving); any psum row
            # = colsums. One accumulation group per bank; the pool closes
            # (banks freed) before the later psum pools open. The colsum
            # chunks drain straight to DRAM (cc_in, p-major) for the
            # collective — no SBUF bounce.
            cc_in = dram.tile([C], f32, name="cc_in")
            cc_out = dram.tile([N], f32, name="cc_out")
            last_cs_mm = None
            with tc.tile_pool(name="degpsum", bufs=1, space="PSUM") as degpsum:
                deg_ps = [
                    degpsum.tile([128, DEG_W], f32, name=f"deg_ps{h}")
                    for h in range(DEG_CH)
                ]
                for kp in range(NP):
                    for h in range(DEG_CH):
                        last_cs_mm = nc.tensor.matmul(
                            deg_ps[h][:],
                            ones_v[:, :, 0:128],
                            a_pair(kp, h * DEG_W, (h + 1) * DEG_W),
                            start=(kp == 0),
                            stop=(kp == NP - 1),
                            perf_mode=DR,
                        )
                # drain chunk row 0s to SBUF (alternating engines), then one
                # DMA ships all degrees to the DRAM collective input,
                # scattered p-major: cc_in[p*J + j] = deg(j*128 + p).
                deg_sb = persist.tile([1, C], f32, name="deg_sb")
                for h in range(DEG_CH):
                    seg = deg_sb[:, h * DEG_W : (h + 1) * DEG_W]
                    if h % 2 == 0:
                        nc.vector.tensor_copy(seg, deg_ps[h][0:1, :])
                    else:
                        nc.scalar.copy(seg, deg_ps[h][0:1, :])
                nc.scalar.dma_start(
                    out=cc_in.rearrange("(p j) -> j p", p=128),
                    in_=deg_sb[0:1, :],
                )

            # ---- gather degrees (p-major shards so both readbacks are
            # contiguous-ish), d = scaled rsqrt ----
            coll = nc.gpsimd.collective_compute(
                "AllGather",
                mybir.AluOpType.bypass,
                replica_groups=[list(range(NCORES))],
                ins=[cc_in[:]],
                outs=[cc_out[:]],
            )
            deg_full = work.tile([128, KT], f32, tag="deg_full")
            nc.sync.dma_start(
                out=deg_full.rearrange("p (c j) -> p c j", c=NCORES),
                in_=cc_out.rearrange("(c p j) -> p c j", p=128, c=NCORES),
            )
            # ds = S / sqrt(deg): reciprocal then sqrt(S^2 * x)
            ds = persist.tile([128, KT], f32, name="ds")
            ds_recip = nc.vector.reciprocal(ds[:], deg_full[:])
            nc.scalar.activation(
                ds[:], ds[:], mybir.ActivationFunctionType.Sqrt, scale=S * S
            )

            # d_own' = (1/S) / sqrt(deg_own), from the local (pre-gather) degs
            deg_own = work.tile([128, J], f32, tag="deg_own")
            nc.sync.dma_start(
                out=deg_own[:], in_=cc_in.rearrange("(p j) -> p j", p=128)
            )
            d_own = persist.tile([128, J], f32, name="d_own")
            nc.vector.reciprocal(d_own[:], deg_own[:])
            nc.scalar.activation(
                d_own[:],
                d_own[:],
                mybir.ActivationFunctionType.Sqrt,
                scale=1.0 / (S * S),
            )

            # ---- Z = X @ W (fp16 in, f32 accum, fp16 out), unscaled.
            # No degree dependency: fills the collective's latency window.
            # 4 psum bufs so the PE never stalls on the psum->SBUF drains.
            z_sb = [persist.tile([128, F], fp16, name=f"z{m}") for m in range(KT)]
            first_z_mm = None
            last_xw_mm = None
            with tc.tile_pool(name="zpsum", bufs=6, space="PSUM") as zpsum:
                for ch in range(XCH):
                    for mi in range(MPC):
                        m = ch * MPC + mi
                        z_ps = zpsum.tile([128, F], f32, tag="z_ps")
                        for i in range(2):
                            mm = nc.tensor.matmul(
                                z_ps[:],
                                xt_tiles[(ch, i)][:, mi * 128 : (mi + 1) * 128],
                                w_sb[i][:],
                                start=(i == 0),
                                stop=(i == 1),
                            )
                            last_xw_mm = mm
                            if first_z_mm is None:
                                first_z_mm = mm
                                # keep the Z matmuls behind the colsum chain
                                # on the PE queue (colsum gates the
                                # collective -> critical path)
                                bass._add_dep_helper(
                                    mm.ins, last_cs_mm.ins, reason="z after colsum"
                                )
                        if m % 2 == 0:
                            nc.vector.tensor_copy(z_sb[m][:], z_ps[:])
                        else:
                            nc.scalar.copy(z_sb[m][:], z_ps[:])

            # ---- fp8 hi/lo split of Zd = ds * Z, two fused ops per tile:
            #   hi = fp8(z * ds)        (ACT activation-with-scale / DVE
            #                            tensor_scalar, fp8 out)
            #   lo = fp8(z * ds - hi)   (DVE/Pool scalar_tensor_tensor)
            # rotated so no single engine becomes the pipeline bottleneck.
            zhi = [persist.tile([128, 2 * F], fp8, name=f"zh{kp}") for kp in range(NP)]
            zlo = [persist.tile([128, 2 * F], fp8, name=f"zl{kp}") for kp in range(NP)]
            for m in range(KT):
                kp, half = divmod(m, 2)
                hi = zhi[kp][:, half * F : (half + 1) * F]
                lo = zlo[kp][:, half * F : (half + 1) * F]
                dsm = ds[:, m : m + 1]
                nc.scalar.activation(
                    hi,
                    z_sb[m][:],
                    mybir.ActivationFunctionType.Copy,
                    scale=dsm,
                )
                # walrus: TensorScalarPtr is DVE-only (Pool rejected at
                # codegen) -> all lo ops ride DVE
                nc.vector.scalar_tensor_tensor(
                    lo,
                    z_sb[m][:],
                    dsm,
                    hi,
                    mybir.AluOpType.mult,
                    mybir.AluOpType.subtract,
                )

            # ---- big matmul: out[j, f] += A[i, j] * Zd[i, f] in fp8
            # DoubleRow, k-outer / jt-inner: 8 simultaneous accumulation
            # groups, one per PSUM bank, consuming hi/lo pairs as the
            # conversion pipeline produces them.
            with tc.tile_pool(name="outpsum", bufs=1, space="PSUM") as outpsum:
                out_ps = [
                    outpsum.tile([128, F], f32, name=f"ops{jt}") for jt in range(JT)
                ]
                # PE p-state warm-up across the PE-idle gap between the
                # collective's end and the first (degree-gated) fp8 pair:
                # one dummy chain fires at the collective's completion, a
                # second at the ds reciprocal, so the PE is at full clock
                # when the real pairs arrive (they queue behind on the PE
                # FIFO, so overshoot would delay the MM — keep them short).
                for i in range(48):
                    dmm = nc.tensor.matmul(
                        out_ps[0][:, 0:128],
                        ones_v[:, :, 0:128],
                        ones_v[:, :, 0:128],
                        start=True,
                        stop=True,
                        perf_mode=DR,
                    )
                    dep = coll if i < 24 else ds_recip
                    bass._add_dep_helper(dmm.ins, dep.ins, reason="pe warmup")
                for kp in range(NP):
                    zh = zhi[kp].rearrange("p (t f) -> p t f", t=2)
                    zl = zlo[kp].rearrange("p (t f) -> p t f", t=2)
                    for jt in range(JT):
                        ap = a_pair(kp, jt * 128, (jt + 1) * 128)
                        nc.tensor.matmul(
                            out_ps[jt][:],
                            ap,
                            zh,
                            start=(kp == 0),
                            stop=False,
                            perf_mode=DR,
                        )
                        nc.tensor.matmul(
                            out_ps[jt][:],
                            ap,
                            zl,
                            start=False,
                            stop=(kp == NP - 1),
                            perf_mode=DR,
                        )
                        if kp == NP - 1:
                            # epilogue pipelined into the final pair:
                            # out = d_own' * psum + bias, spread across
                            # DVE (fused) and ACT+Pool (scale, then add)
                            # so the tail drains in parallel.
                            # early jts (which stop first) take the longer
                            # ACT+Pool chain; later jts the short DVE one.
                            # Stores 5/3 across SP/ACT so no single store
                            # lane serializes the final drain.
                            ot = work.tile([128, F], f32, tag="ot", bufs=8)
                            if jt % 2 == 1:
                                nc.vector.scalar_tensor_tensor(
                                    ot[:],
                                    out_ps[jt][:],
                                    d_own[:, jt : jt + 1],
                                    bias_sb[:],
                                    mybir.AluOpType.mult,
                                    mybir.AluOpType.add,
                                )
                            else:
                                sc = work.tile([128, F], f32, tag="sc", bufs=4)
                                nc.scalar.activation(
                                    sc[:],
                                    out_ps[jt][:],
                                    mybir.ActivationFunctionType.Copy,
                                    scale=d_own[:, jt : jt + 1],
                                )
                                nc.gpsimd.tensor_tensor(
                                    ot[:], sc[:], bias_sb[:], mybir.AluOpType.add
                                )
                            eng = nc.sync if jt % 2 == 1 else nc.scalar
                            eng.dma_start(
                                out=out[jt * 128 : (jt + 1) * 128, :], in_=ot[:]
                            )
    if split_waits:
        _split_dma_waits(nc)
    return nc


_CACHE = {}


def _get_program(n_nodes: int, debug: bool = False):
    key = (n_nodes, debug)
    if key not in _CACHE:
        _CACHE[key] = build(n_nodes, debug=debug)
    return _CACHE[key]


def _prep_inputs(A, inputs, weight, bias):
    """Host-side marshaling: shard + layout + dtype casts."""
    N = A.shape[0]
    C = N // NCORES
    KT = N // 128
    A_aug = np.asarray(A, dtype=np.float32)
    idx = np.arange(N)
    A_aug = A_aug.astype(ml_dtypes.float8_e4m3)
    A_aug[idx, idx] = np.float32(1.0)  # reference adds I; A diag is 0
    xt = np.ascontiguousarray(np.asarray(inputs, dtype=np.float32).T).astype(
        np.float16
    )
    w16 = np.asarray(weight, dtype=np.float32).astype(np.float16)
    bias_bc = np.ascontiguousarray(
        np.broadcast_to(np.asarray(bias, dtype=np.float32), (128, F))
    )
    in_maps = []
    for c in range(NCORES):
        strip = A_aug[:, c * C : (c + 1) * C]  # [N, C]
        # partition-major pre-tiling: [128, KT*C]
        tiled = np.ascontiguousarray(
            strip.reshape(KT, 128, C).transpose(1, 0, 2).reshape(128, KT * C)
        )
        in_maps.append(
            {"a_strip": tiled, "xt": xt, "w": w16, "bias_bc": bias_bc}
        )
    return in_maps


def kernel(A, inputs, weight, bias):
    N = A.shape[0]
    nc = _get_program(N)
    in_maps = _prep_inputs(A, inputs, weight, bias)
    res = run_bass_kernel_spmd(nc, in_maps, list(range(NCORES)))
    return np.concatenate([r["out"] for r in res.results], axis=0)


if __name__ == "__main__":
    # mini self-check with a host reference
    N = 1024
    rng = np.random.default_rng(0)
    A = (rng.random((N, N)) < 0.01).astype(np.float32)
    A = np.maximum(A, A.T)
    np.fill_diagonal(A, 0.0)
    X = rng.standard_normal((N, F)).astype(np.float32)
    W = (rng.random((F, F)).astype(np.float32) / 100.0) - 0.005
    b = (rng.random(F).astype(np.float32) / 100.0) - 0.005

    A_ = A + np.eye(N, dtype=np.float32)
    deg = A_.sum(axis=1)
    d = deg**-0.5
    expected = (d[:, None] * A_ * d[None, :]) @ X @ W + b

    nc = _get_program(N)
    in_maps = _prep_inputs(A, X, W, b)
    res = run_bass_kernel_spmd(nc, in_maps, list(range(NCORES)))
    got = np.concatenate([r["out"] for r in res.results], axis=0)
    err = np.abs(got - expected)
    scale = np.abs(expected).max()
    print("rel err:", err.max() / scale, "nan:", np.isnan(got).sum(), "/", got.size)
